# revision 1
# baseline (speedup 1.0000x reference)
"""DoubleAttention Trainium2 kernel — data-parallel over batch across 8 cores.

Self-contained: takes full inputs, shards n=16 over 8 cores (2 samples/core),
runs a Bass/Tile kernel per core, gathers the full output.

Math per sample (C=512, KC=256, VC=512, H=8 heads, L=4096):
  K = Wk@X, Q = Wq@X, V = Wv@X          (1x1 convs as matmuls)
  key_sm = softmax_L(K + bk) = softmax_L(K)        (bk shift-invariant)
  q_sm   = softmax_head32(Q + bq)
  context_h = V_h @ key_sm_h^T                      (per head, 64x32)
  att = context @ q_sm (block-diag) ; out = x + We@att + be
Folds used by the kernel:
  - context^T computed directly as E_k^T-lhsT matmuls (E=exp(K), layout B)
  - 1/sum_k and per-head block masking folded into context_n
  - M^T = (We @ context_n)^T precomputed once per sample -> output proj
    contracts over 256 (not 512) and absorbs the att matmul
  - bv, be folded into wbias = We@bv + be (host) ; bk dropped (no-op)
All big matmuls in float32r (full PE rate, ~1e-3 rel err).
Sample 1 stage-1 is interleaved with sample 0 phase-A to keep PE dense.
"""

import numpy as np

_CACHE = {}

N_CORES = 8
N, C, Hdim, Wdim = 16, 512, 64, 64
L = Hdim * Wdim            # 4096
KC, VC = 256, 512
NH = 8                     # heads
HV = VC // NH              # 64 head value channels
S_PER_CORE = N // N_CORES  # 2 samples per core
NB = L // 512              # 8 L-banks of 512
NT = L // 128              # 32 L-tiles of 128


def _build_nc():
    import concourse.mybir as mybir
    import concourse.tile as tile
    from concourse import bacc

    F32 = mybir.dt.float32
    F32R = mybir.dt.float32r
    AF = mybir.ActivationFunctionType
    ALU = mybir.AluOpType

    nc = bacc.Bacc("TRN2", target_bir_lowering=False, debug=False)

    # f32r dram views: same bits as f32, lets HWDGE (sync) DMA load without
    # a casting (gpsimd-only) path; PE rounds on consumption.
    xin = nc.dram_tensor("xin", [S_PER_CORE * C, L], F32R, kind="ExternalInput")
    wkT_d = nc.dram_tensor("wkT", [128, 4, KC], F32R, kind="ExternalInput")
    wqT_d = nc.dram_tensor("wqT", [128, 4, KC], F32R, kind="ExternalInput")
    wvT_d = nc.dram_tensor("wvT", [128, 4, VC], F32R, kind="ExternalInput")
    weT_d = nc.dram_tensor("weT", [128, 4, C], F32R, kind="ExternalInput")
    bq_d = nc.dram_tensor("bqv", [128, 2], F32, kind="ExternalInput")
    wb_d = nc.dram_tensor("wbv", [128, 4], F32, kind="ExternalInput")
    g4_d = nc.dram_tensor("g4", [128, 4], F32R, kind="ExternalInput")
    bs_d = nc.dram_tensor("bsum", [128, 128], F32R, kind="ExternalInput")
    idr_d = nc.dram_tensor("identr", [128, 128], F32R, kind="ExternalInput")
    id_d = nc.dram_tensor("ident", [128, 128], F32, kind="ExternalInput")
    ones_d = nc.dram_tensor("ones", [128, 1], F32R, kind="ExternalInput")
    out_d = nc.dram_tensor("out", [S_PER_CORE * C, L], F32, kind="ExternalOutput")

    with tile.TileContext(nc) as tc:
        with tc.tile_pool(name="wpool", bufs=1) as wp, \
             tc.tile_pool(name="work", bufs=1) as sp, \
             tc.tile_pool(name="ppool", bufs=1, space="PSUM") as pp:

            # ---- resident weights/constants ----
            wkT = wp.tile([128, 4, KC], F32R, name="wkT_s")
            wqT = wp.tile([128, 4, KC], F32R, name="wqT_s")
            wvT = wp.tile([128, 4, VC], F32R, name="wvT_s")
            weT = wp.tile([128, 4, C], F32R, name="weT_s")
            g4 = wp.tile([128, 4], F32R, name="g4_s")
            bsum = wp.tile([128, 128], F32R, name="bsum_s")
            idr = wp.tile([128, 128], F32R, name="idr_s")
            ident = wp.tile([128, 128], F32, name="id_s")
            ones = wp.tile([128, 1], F32R, name="ones_s")
            bq = wp.tile([128, 2], F32, name="bq_s")
            wb = wp.tile([128, 4], F32, name="wb_s")

            # first x tile goes out before the weights so DMA queues overlap
            x4_first = sp.tile([128, 4, 512], F32R, name="x4_0_0", tag="x4",
                               bufs=4)
            for c in range(4):
                nc.gpsimd.dma_start(
                    out=x4_first[:, c, :],
                    in_=xin[c * 128:(c + 1) * 128, 0:512])
                nc.sync.dma_start(out=wkT[:, c, :], in_=wkT_d[:, c, :])
                nc.sync.dma_start(out=wvT[:, c, :], in_=wvT_d[:, c, :])
            for dst, src in ((wqT, wqT_d), (weT, weT_d), (g4, g4_d),
                             (bsum, bs_d), (idr, idr_d), (ident, id_d),
                             (ones, ones_d), (bq, bq_d), (wb, wb_d)):
                nc.sync.dma_start(out=dst, in_=src[...])

            st = {}   # per-sample state: ctx_ps, skt_ps, mt

            def stage1_group(s, g, ks=range(4)):
                row0 = s * C
                if g == 0:
                    st[s] = dict(
                        ctx_ps=[pp.tile([128, KC], F32, name=f"ctx{s}_{j}",
                                        tag="ctx", bufs=2) for j in range(2)],
                        skt_ps=pp.tile([1, KC], F32, name=f"skt{s}",
                                       tag="skt", bufs=2))
                if s == 0 and g == 0:
                    x4 = x4_first
                else:
                    x4 = sp.tile([128, 4, 512], F32R, name=f"x4_{s}_{g}",
                                 tag="x4", bufs=4)
                    nc.gpsimd.dma_start(
                        out=x4,
                        in_=xin[row0:row0 + C, g * 512:(g + 1) * 512]
                        .rearrange("(c p) l -> p c l", p=128))
                ctx_ps, skt_ps = st[s]["ctx_ps"], st[s]["skt_ps"]
                st[s]["x4cur"] = x4
                for k in ks:
                    t = 4 * g + k
                    ksl = slice(k * 128, (k + 1) * 128)
                    kt_ps = pp.tile([128, KC], F32, name=f"kt{s}_{t}",
                                    tag="kt", bufs=2)
                    vt_ps = pp.tile([128, VC], F32, name=f"vt{s}_{t}",
                                    tag="vt", bufs=2)
                    # kt/vt interleaved: consecutive matmuls share the same
                    # stationary operand x4[:, c, ksl]
                    for c in range(4):
                        nc.tensor.matmul(kt_ps[:, :], x4[:, c, ksl],
                                         wkT[:, c, :],
                                         start=(c == 0), stop=(c == 3))
                        nc.tensor.matmul(vt_ps[:, :], x4[:, c, ksl],
                                         wvT[:, c, :],
                                         start=(c == 0), stop=(c == 3))
                    ekT = sp.tile([128, KC], F32R, name=f"ek{s}_{t}",
                                  tag="ek", bufs=6)
                    nc.scalar.activation(ekT[:, :], kt_ps[:, :], AF.Exp)
                    vt = sp.tile([128, VC], F32R, name=f"vts{s}_{t}",
                                 tag="vts", bufs=6)
                    nc.vector.tensor_copy(vt[:, :], vt_ps[:, :])
                    for j in range(2):
                        jsl = slice(j * 128, (j + 1) * 128)
                        nc.tensor.matmul(ctx_ps[j][:, :], ekT[:, jsl],
                                         vt[:, j * KC:(j + 1) * KC],
                                         start=(t == 0), stop=(t == NT - 1))
                    nc.tensor.matmul(skt_ps[:, :], ones[:, :], ekT[:, :],
                                     start=(t == 0), stop=(t == NT - 1))

            def stage1_tiles2(s, g):
                # second half of a group, x4 already loaded by first half
                row0 = s * C
                x4 = st[s]["x4cur"]
                ctx_ps, skt_ps = st[s]["ctx_ps"], st[s]["skt_ps"]
                for k in (2, 3):
                    t = 4 * g + k
                    ksl = slice(k * 128, (k + 1) * 128)
                    kt_ps = pp.tile([128, KC], F32, name=f"ktb{s}_{t}",
                                    tag="kt", bufs=2)
                    vt_ps = pp.tile([128, VC], F32, name=f"vtb{s}_{t}",
                                    tag="vt", bufs=2)
                    for c in range(4):
                        nc.tensor.matmul(kt_ps[:, :], x4[:, c, ksl],
                                         wkT[:, c, :],
                                         start=(c == 0), stop=(c == 3))
                        nc.tensor.matmul(vt_ps[:, :], x4[:, c, ksl],
                                         wvT[:, c, :],
                                         start=(c == 0), stop=(c == 3))
                    ekT = sp.tile([128, KC], F32R, name=f"ekb{s}_{t}",
                                  tag="ek", bufs=6)
                    nc.scalar.activation(ekT[:, :], kt_ps[:, :], AF.Exp)
                    vt = sp.tile([128, VC], F32R, name=f"vtsb{s}_{t}",
                                 tag="vts", bufs=6)
                    nc.vector.tensor_copy(vt[:, :], vt_ps[:, :])
                    for j in range(2):
                        jsl = slice(j * 128, (j + 1) * 128)
                        nc.tensor.matmul(ctx_ps[j][:, :], ekT[:, jsl],
                                         vt[:, j * KC:(j + 1) * KC],
                                         start=(t == 0), stop=(t == NT - 1))
                    nc.tensor.matmul(skt_ps[:, :], ones[:, :], ekT[:, :],
                                     start=(t == 0), stop=(t == NT - 1))

            def mid(s):
                ctx_ps, skt_ps = st[s]["ctx_ps"], st[s]["skt_ps"]
                sk_sb = sp.tile([1, KC], F32, name=f"sksb{s}", tag="sksb",
                                bufs=2)
                nc.vector.tensor_copy(sk_sb[:, :], skt_ps[:, :])
                # transpose the (1,256) sum row into (128,2) via two K=1
                # matmuls against a 1x1 identity (lhsT.T @ [1] = column)
                rk_ps = pp.tile([128, 2], F32, name=f"rkps{s}", tag="skt",
                                bufs=2)
                for j in range(2):
                    nc.tensor.matmul(rk_ps[:, j:j + 1],
                                     sk_sb[0:1, j * 128:(j + 1) * 128],
                                     ident[0:1, 0:1],
                                     start=True, stop=True)
                rk = sp.tile([128, 2], F32, name=f"rk{s}", tag="rk", bufs=2)
                nc.vector.reciprocal(rk[:, :], rk_ps[:, :])
                ctn = sp.tile([128, 2, KC], F32, name=f"ctn{s}", tag="ctn",
                              bufs=2)
                nc.vector.memset(ctn[:, :, :], 0.0)
                for h in range(NH):
                    j, gg = h // 4, h % 4
                    pr = slice(32 * gg, 32 * gg + 32)
                    vr = slice(HV * gg, HV * gg + HV)   # local v within chunk
                    nc.vector.tensor_scalar_mul(
                        ctn[pr, j, vr], ctx_ps[j][pr, vr], rk[pr, j:j + 1])
                tr_ps = [pp.tile([128, KC], F32, name=f"tr{s}_{j}", tag="kt",
                                 bufs=2) for j in range(2)]
                for j in range(2):
                    for vcl in range(2):
                        vsl = slice(vcl * 128, (vcl + 1) * 128)
                        nc.tensor.transpose(tr_ps[j][:, vsl], ctn[:, j, vsl],
                                            ident[:, :])
                cn = sp.tile([128, 2, KC], F32R, name=f"cn{s}", tag="cn",
                             bufs=2)
                for j in range(2):
                    jsl = slice(j * 128, (j + 1) * 128)
                    nc.scalar.copy(
                        cn[:, :, jsl],
                        tr_ps[j][:, :].rearrange("p (v q) -> p v q", v=2))
                mt = sp.tile([128, 2, C], F32R, name=f"mt{s}", tag="mt",
                             bufs=2)
                for j in range(2):
                    jsl = slice(j * 128, (j + 1) * 128)
                    mt_ps = pp.tile([128, C], F32, name=f"mtp{s}_{j}",
                                    tag="vt", bufs=2)
                    for vcl in range(2):
                        nc.tensor.matmul(mt_ps[:, :], cn[:, vcl, jsl],
                                         weT[:, 2 * j + vcl, :],
                                         start=(vcl == 0), stop=(vcl == 1))
                    nc.scalar.copy(mt[:, j, :], mt_ps[:, :])
                st[s]["mt"] = mt

            # phase A is software-pipelined: the softmax chain of bank
            # b+1 is emitted before the output stage of bank b, so ACT's
            # FIFO never queues exp() behind oc copies and PE always has
            # matmul work while the DVE/ACT chain completes.
            pend = []

            def _softmaxA(s, b):
                row0 = s * C
                bsl = slice(b * 512, (b + 1) * 512)
                xb = sp.tile([128, 4, 512], F32R, name=f"xb{s}_{b}", tag="xb",
                             bufs=3)
                nc.gpsimd.dma_start(
                    out=xb,
                    in_=xin[row0:row0 + C, bsl]
                    .rearrange("(c p) l -> p c l", p=128))
                eqs = []
                for j in range(2):
                    jsl = slice(j * 128, (j + 1) * 128)
                    q_ps = pp.tile([128, 512], F32, name=f"q{s}_{b}_{j}",
                                   tag="kt", bufs=2)
                    for c in range(4):
                        nc.tensor.matmul(q_ps[:, :], wqT[:, c, jsl],
                                         xb[:, c, :],
                                         start=(c == 0), stop=(c == 3))
                    eq = sp.tile([128, 512], F32R, name=f"eq{s}_{b}_{j}",
                                 tag="eq", bufs=4)
                    nc.scalar.activation(eq[:, :], q_ps[:, :], AF.Exp,
                                         bias=bq[:, j:j + 1])
                    eqs.append(eq)
                qsm = []
                for j in range(2):
                    sq_ps = pp.tile([128, 512], F32, name=f"sq{s}_{b}_{j}",
                                    tag="skt", bufs=2)
                    nc.tensor.matmul(sq_ps[:, :], bsum[:, :], eqs[j][:, :],
                                     start=True, stop=True)
                    rf = sp.tile([128, 512], F32, name=f"rf{s}_{b}_{j}",
                                 tag="rf", bufs=3)
                    nc.vector.reciprocal_approx_fast(rf[:, :], sq_ps[:, :])
                    qs = sp.tile([128, 512], F32R, name=f"qs{s}_{b}_{j}",
                                 tag="qs", bufs=4)
                    nc.vector.tensor_mul(qs[:, :], eqs[j][:, :], rf[:, :])
                    qsm.append(qs)
                return xb, qsm

            def _outputA(s, b, xb, qsm):
                row0 = s * C
                mt = st[s]["mt"]
                bsl = slice(b * 512, (b + 1) * 512)
                for c in range(4):
                    o_ps = pp.tile([128, 512], F32, name=f"o{s}_{b}_{c}",
                                   tag="vt", bufs=2)
                    csl = slice(c * 128, (c + 1) * 128)
                    if c < 2:
                        # residual + wbias on DVE (PE array is the bottleneck)
                        for j in range(2):
                            nc.tensor.matmul(o_ps[:, :], mt[:, j, csl],
                                             qsm[j][:, :],
                                             start=(j == 0), stop=(j == 1))
                        oc = sp.tile([128, 512], F32, name=f"oc{s}_{b}_{c}",
                                     tag="oc", bufs=4)
                        nc.vector.scalar_tensor_tensor(
                            out=oc[:, :], in0=o_ps[:, :],
                            scalar=wb[:, c:c + 1],
                            in1=xb[:, c, :].bitcast(F32),
                            op0=ALU.add, op1=ALU.add)
                    else:
                        # residual folded into PSUM via identity matmul,
                        # wbias via ACT bias-add
                        for j in range(2):
                            nc.tensor.matmul(o_ps[:, :], mt[:, j, csl],
                                             qsm[j][:, :],
                                             start=(j == 0), stop=False)
                        nc.tensor.matmul(o_ps[:, :], idr[:, :], xb[:, c, :],
                                         start=False, stop=True)
                        oc = sp.tile([128, 512], F32, name=f"oc{s}_{b}_{c}",
                                     tag="oc", bufs=4)
                        nc.scalar.add(oc[:, :], o_ps[:, :], wb[:, c:c + 1])
                    nc.sync.dma_start(
                        out=out_d[row0 + c * 128:row0 + (c + 1) * 128, bsl],
                        in_=oc[:, :])

            def phaseA_bank(s, b):
                pend.append((s, b) + _softmaxA(s, b))
                if len(pend) > 1:
                    _outputA(*pend.pop(0))

            def phaseA_flush():
                while pend:
                    _outputA(*pend.pop(0))

            # schedule: keep PE dense by interleaving independent work:
            # s1 stage-1 overlaps mid(0)+phaseA(0); leftover phaseA(0) banks
            # are spread through phaseA(1) so chain stalls are filled.
            for g in range(NB):
                stage1_group(0, g)
            stage1_group(1, 0)
            stage1_group(1, 1)
            mid(0)
            # interleave: s1 stage-1 with the first half of s0 phase-A;
            # defer 4 s0 banks into the tail so it always has two
            # independent chains to alternate between.
            for i in range(2, NB):
                if i >= 4:
                    stage1_group(1, i, ks=(0, 1))
                    pend.append((0, i - 4) + _softmaxA(0, i - 4))
                    stage1_tiles2(1, i)
                    if len(pend) > 1:
                        _outputA(*pend.pop(0))
                else:
                    stage1_group(1, i)
            mid(1)
            phaseA_bank(0, 4)
            for i in range(NB):
                phaseA_bank(1, i)
                if i in (0, 2, 4):
                    phaseA_bank(0, 5 + i // 2)
            phaseA_flush()
    nc.compile()
    return nc


def _host_prep(Wk, bk, Wq, bq, Wv, bv, We, be):
    f = np.float32
    def chunkT(w, nchunk):          # (O, C) -> lhsT layout (128, nchunk, O)
        wt = np.ascontiguousarray(w.T.astype(f))          # (C, O)
        return np.ascontiguousarray(
            wt.reshape(nchunk, 128, w.shape[0]).transpose(1, 0, 2))
    wkT = chunkT(Wk, 4)             # (128, 4, 256)
    wqT = chunkT(Wq, 4)
    wvT = chunkT(Wv, 4)
    weT = chunkT(We, 4)             # We.T chunks over v -> (128, 4, 512)
    bq2 = np.ascontiguousarray(bq.astype(f).reshape(2, 128).T)
    wb = np.ascontiguousarray(
        (We.astype(np.float64) @ bv.astype(np.float64)
         + be.astype(np.float64)).astype(f).reshape(4, 128).T)
    g4 = np.zeros((128, 4), f)
    for p in range(128):
        g4[p, p // 32] = 1.0
    bsum = np.zeros((128, 128), f)
    for p in range(128):
        bsum[p, (p // 32) * 32:(p // 32) * 32 + 32] = 1.0
    ident = np.eye(128, dtype=f)
    ones = np.ones((128, 1), f)
    return dict(wkT=wkT, wqT=wqT, wvT=wvT, weT=weT, bqv=bq2, wbv=wb,
                g4=g4, bsum=bsum, identr=ident, ident=ident,
                ones=ones)


def kernel(x, Wk, bk, Wq, bq, Wv, bv, We, be):
    from concourse.bass_utils import run_bass_kernel_spmd

    assert x.shape == (N, C, Hdim, Wdim), x.shape
    if "nc" not in _CACHE:
        _CACHE["nc"] = _build_nc()
    nc = _CACHE["nc"]

    shared = _host_prep(Wk, bk, Wq, bq, Wv, bv, We, be)
    xf = np.ascontiguousarray(x.astype(np.float32).reshape(N, C, L))
    in_maps = []
    for i in range(N_CORES):
        m = dict(shared)
        m["xin"] = np.ascontiguousarray(
            xf[i * S_PER_CORE:(i + 1) * S_PER_CORE].reshape(S_PER_CORE * C, L))
        in_maps.append(m)

    res = run_bass_kernel_spmd(nc, in_maps, core_ids=list(range(N_CORES)))
    out = np.concatenate(
        [res.results[i]["out"].reshape(S_PER_CORE, C, Hdim, Wdim)
         for i in range(N_CORES)], axis=0)
    return out.astype(np.float32)



# revision 3
# speedup vs baseline: 1.6796x; 1.6796x over previous
"""DoubleAttention Trainium2 kernel — data-parallel over batch across 8 cores.

Self-contained: takes full inputs, shards n=16 over 8 cores (2 samples/core),
runs a Bass/Tile kernel per core, gathers the full output.

Math per sample (C=512, KC=256, VC=512, H=8 heads, L=4096):
  K = Wk@X, Q = Wq@X, V = Wv@X          (1x1 convs as matmuls)
  key_sm = softmax_L(K)  (bk shift cancels in the ratio)
  q_sm   = softmax_head32(Q + bq)
  ctx_h = V_h @ key_sm_h^T (per head) ; att = ctx @ q_sm (block-diag)
  out = x + We@att + (We@bv + be)

This version:
  - x, Wk, Wq, Wv are cast to fp8(e4m3) on the HOST; all big matmuls run
    as fp8 DoubleRow (K=256 per instruction, 2x fp32r FLOP rate).
  - exp(K) emitted directly as fp8 from the ACT engine (scale=1/8 undoes
    the 8x weight prescale; a host-computed shift keeps E under ~240).
  - skt (softmax-L denominators) folded into the ctx matmul as a ones
    column of the V tile; lands as column 256 of ctx PSUM per k-row.
  - q softmax: single bf16 exp (bq folded into the group-sum matrix and
    the mt8 scale), bf16 group-sum matmul, DVE reciprocal + multiply.
  - residual x and the (We@bv + be) bias are added on the HOST; device
    returns the attention term only, in bf16 (halves output DMA).
"""

import numpy as np
import ml_dtypes

_CACHE = {}

N_CORES = 8
N, C, Hdim, Wdim = 16, 512, 64, 64
L = Hdim * Wdim            # 4096
KC, VC = 256, 512
NH = 8
S_PER_CORE = N // N_CORES  # 2
NB = L // 512              # 8 banks of 512
NPAIR = L // 256           # 16 l-tile pairs (tiles of 128)
WS = 8.0                   # weight prescale, undone by exp scale=1/8

F8NP = ml_dtypes.float8_e4m3
BFNP = ml_dtypes.bfloat16


def _build_nc():
    import concourse.mybir as mybir
    import concourse.tile as tile
    from concourse import bacc

    F32 = mybir.dt.float32
    F32R = mybir.dt.float32r
    FP8 = mybir.dt.float8e4
    BF = mybir.dt.bfloat16
    AF = mybir.ActivationFunctionType
    DR = mybir.MatmulPerfMode.DoubleRow

    nc = bacc.Bacc("TRN2", target_bir_lowering=False, debug=False)

    x8_d = nc.dram_tensor("x8", [S_PER_CORE * C, L], FP8, kind="ExternalInput")
    wk_d = nc.dram_tensor("wk8", [128, 4, KC], FP8, kind="ExternalInput")
    wq_d = nc.dram_tensor("wq8", [128, 4, KC], FP8, kind="ExternalInput")
    wv_d = nc.dram_tensor("wv8", [128, 4, VC], FP8, kind="ExternalInput")
    we_d = nc.dram_tensor("weT", [128, 4, C], F32R, kind="ExternalInput")
    eb_d = nc.dram_tensor("ebq", [128, 2], F32, kind="ExternalInput")
    ks_d = nc.dram_tensor("ksh", [128, 1], F32, kind="ExternalInput")
    bs_d = nc.dram_tensor("bsum", [128, 2, 128], BF, kind="ExternalInput")
    id_d = nc.dram_tensor("ident", [128, 128], F32, kind="ExternalInput")
    out_d = nc.dram_tensor("out", [S_PER_CORE * C, L], BF,
                           kind="ExternalOutput")

    with tile.TileContext(nc) as tc:
        with tc.tile_pool(name="wpool", bufs=1) as wp, \
             tc.tile_pool(name="work", bufs=1) as sp, \
             tc.tile_pool(name="ppool", bufs=1, space="PSUM") as pp:

            # ---- resident tensors ----
            x8 = wp.tile([128, 2 * 4, L], FP8, name="x8s")
            wk8 = wp.tile([128, 4, KC], FP8, name="wk8s")
            wq8 = wp.tile([128, 4, KC], FP8, name="wq8s")
            wv8 = wp.tile([128, 4, VC], FP8, name="wv8s")
            weT = wp.tile([128, 4, C], F32R, name="weTs")
            ebq = wp.tile([128, 2], F32, name="ebqs")
            ksh = wp.tile([128, 1], F32, name="kshs")
            bsum = wp.tile([128, 2, 128], BF, name="bsums")
            ident = wp.tile([128, 128], F32, name="ids")
            # V pair tiles (ping-pong): layout [128, pair, j*257+q]; col 256
            # of each j-block is the ones column that accumulates skt.
            vtp = [wp.tile([128, 2, 2 * 257], FP8, name=f"vtp{i}")
                   for i in range(2)]

            def load_group(s, g):
                gsl = slice(g * 512, (g + 1) * 512)
                nc.gpsimd.dma_start(
                    out=x8[:, 4 * s:4 * s + 4, gsl],
                    in_=x8_d[s * C:(s + 1) * C, gsl]
                    .rearrange("(c p) l -> p c l", p=128))

            load_group(0, 0)
            for dst, src in ((wk8, wk_d), (wv8, wv_d), (wq8, wq_d),
                             (weT, we_d), (ebq, eb_d), (ksh, ks_d),
                             (bsum, bs_d), (ident, id_d)):
                nc.sync.dma_start(out=dst, in_=src[...])
            for i in range(2):
                for j in range(2):
                    nc.vector.memset(
                        vtp[i][:, :, j * 257 + 256:j * 257 + 257], 1.0)
            for g in range(1, NB):
                load_group(0, g)
            for g in range(NB):
                load_group(1, g)

            st = {0: {}, 1: {}}

            def s1_pair(s, p):
                # one l-pair of stage 1: K^T/V^T fp8-DR matmuls, exp, V copy;
                # the ctx accumulation of pair p-1 is emitted after (pend).
                ktp = pp.tile([128, 2, KC], F32, name=f"kt{s}_{p}",
                              tag="sm", bufs=2)
                vtq = pp.tile([128, 2, VC], F32, name=f"vt{s}_{p}",
                              tag="big", bufs=2)
                for i in range(2):
                    t = 2 * p + i
                    tsl = slice(t * 128, (t + 1) * 128)
                    for cp in range(2):
                        xs = x8[:, 4 * s + 2 * cp:4 * s + 2 * cp + 2, tsl]
                        nc.tensor.matmul(ktp[:, i, :], xs,
                                         wk8[:, 2 * cp:2 * cp + 2, :],
                                         start=(cp == 0), stop=(cp == 1),
                                         perf_mode=DR)
                        nc.tensor.matmul(vtq[:, i, :], xs,
                                         wv8[:, 2 * cp:2 * cp + 2, :],
                                         start=(cp == 0), stop=(cp == 1),
                                         perf_mode=DR)
                ek = sp.tile([128, 2, KC], FP8, name=f"ek{s}_{p}",
                             tag="ek", bufs=3)
                nc.scalar.activation(ek[:, :, :], ktp[:, :, :], AF.Exp,
                                     bias=ksh[:, 0:1], scale=1.0 / WS)
                vts = vtp[p % 2]
                for j in range(2):
                    dst = vts[:, :, j * 257:j * 257 + 256]
                    src = vtq[:, :, j * 256:(j + 1) * 256]
                    if (2 * p + j) % 2 == 0:
                        nc.vector.tensor_copy(dst, src)
                    else:
                        nc.scalar.copy(dst, src)
                return (p, ek, vts)

            def ctx_mm(s, p, ek, vts):
                if "ctx" not in st[s]:
                    st[s]["ctx"] = [
                        pp.tile([128, 257], F32, name=f"ctx{s}_{j}",
                                tag="ctx", bufs=2) for j in range(2)]
                ctx = st[s]["ctx"]
                for j in range(2):
                    nc.tensor.matmul(
                        ctx[j][:, 0:257],
                        ek[:, :, j * 128:(j + 1) * 128],
                        vts[:, :, j * 257:(j + 1) * 257],
                        start=(p == 0), stop=(p == NPAIR - 1),
                        perf_mode=DR)

            def mid_v(s):
                # vector-engine part of the per-sample middle stage
                ctx = st[s]["ctx"]
                rk = sp.tile([128, 2], F32, name=f"rk{s}", tag="rk", bufs=2)
                for j in range(2):
                    nc.vector.reciprocal(rk[:, j:j + 1], ctx[j][:, 256:257])
                ctn = sp.tile([128, 2, KC], F32, name=f"ctn{s}", tag="ctn",
                              bufs=2)
                nc.vector.memset(ctn[:, :, :], 0.0)
                for h in range(NH):
                    j, gg = h // 4, h % 4
                    prr = slice(32 * gg, 32 * gg + 32)
                    vr = slice(64 * gg, 64 * gg + 64)
                    nc.vector.tensor_scalar_mul(
                        ctn[prr, j, vr], ctx[j][prr, vr], rk[prr, j:j + 1])
                st[s]["ctn"] = ctn

            def mid_pe(s):
                # PE part: transpose ctx_n, contract with We^T, cast to fp8
                ctn = st[s]["ctn"]
                tr = [pp.tile([128, 2, KC], F32, name=f"tr{s}_{j}", tag="sm",
                              bufs=2) for j in range(2)]
                for j in range(2):
                    for vcl in range(2):
                        nc.tensor.transpose(
                            tr[j][:, 0, vcl * 128:(vcl + 1) * 128],
                            ctn[:, j, vcl * 128:(vcl + 1) * 128],
                            ident[:, :])
                cn = sp.tile([128, 2, KC], F32R, name=f"cn{s}", tag="cn",
                             bufs=2)
                for j in range(2):
                    nc.scalar.copy(
                        cn[:, :, j * 128:(j + 1) * 128],
                        tr[j][:, 0, :].rearrange("p (v q) -> p v q", v=2))
                mt8 = sp.tile([128, 2, C], FP8, name=f"mt{s}", tag="mt",
                              bufs=2)
                for j in range(2):
                    mtp = pp.tile([128, 2, KC], F32, name=f"mtp{s}_{j}",
                                  tag="sm", bufs=2)
                    mtv = mtp[:, :, :].rearrange("p a b -> p (a b)")
                    for vcl in range(2):
                        nc.tensor.matmul(mtv,
                                         cn[:, vcl, j * 128:(j + 1) * 128],
                                         weT[:, 2 * j + vcl, :],
                                         start=(vcl == 0), stop=(vcl == 1))
                    # fold exp(bq) per kc-row into mt8 during the cast
                    nc.scalar.mul(mt8[:, j, :], mtv, ebq[:, j:j + 1])
                st[s]["mt8"] = mt8

            def phaseA_front(s, b):
                bsl = slice(b * 512, (b + 1) * 512)
                qp = pp.tile([128, 2, 512], F32, name=f"q{s}_{b}", tag="big",
                             bufs=2)
                for j in range(2):
                    for cp in range(2):
                        nc.tensor.matmul(
                            qp[:, j, :],
                            wq8[:, 2 * cp:2 * cp + 2, j * 128:(j + 1) * 128],
                            x8[:, 4 * s + 2 * cp:4 * s + 2 * cp + 2, bsl],
                            start=(cp == 0), stop=(cp == 1), perf_mode=DR)
                eq = sp.tile([128, 2, 512], BF, name=f"eq{s}_{b}", tag="eq",
                             bufs=3)
                nc.scalar.activation(eq[:, :, :], qp[:, :, :], AF.Exp,
                                     scale=1.0 / WS)
                sq = pp.tile([128, 2, 512], F32, name=f"sq{s}_{b}", tag="big",
                             bufs=2)
                for j in range(2):
                    nc.tensor.matmul(sq[:, j, :], bsum[:, j, :], eq[:, j, :],
                                     start=True, stop=True)
                rf = sp.tile([128, 2, 512], F32, name=f"rf{s}_{b}", tag="rf",
                             bufs=3)
                nc.vector.reciprocal_approx_fast(rf[:, :, :], sq[:, :, :])
                qs = sp.tile([128, 2, 512], FP8, name=f"qs{s}_{b}", tag="qs",
                             bufs=3)
                nc.vector.tensor_mul(qs[:, :, :], eq[:, :, :], rf[:, :, :])
                return (s, b, qs)

            def phaseA_out(s, b, qs):
                mt8 = st[s]["mt8"]
                bsl = slice(b * 512, (b + 1) * 512)
                oc = sp.tile([128, 4, 512], BF, name=f"oc{s}_{b}", tag="oc",
                             bufs=3)
                for c in range(4):
                    op = pp.tile([128, 512], F32, name=f"o{s}_{b}_{c}",
                                 tag="ctx", bufs=2)
                    nc.tensor.matmul(op[:, :],
                                     mt8[:, :, c * 128:(c + 1) * 128],
                                     qs[:, :, :], start=True, stop=True,
                                     perf_mode=DR)
                    if c % 2 == 0:
                        nc.vector.tensor_copy(oc[:, c, :], op[:, :])
                    else:
                        nc.scalar.copy(oc[:, c, :], op[:, :])
                nc.gpsimd.dma_start(
                    out=out_d[s * C:(s + 1) * C, bsl]
                    .rearrange("(c p) l -> p c l", p=128),
                    in_=oc[:, :, :])

            # ---- schedule: software-pipelined, mid split so PE never
            # waits on the DVE normalization chain ----
            pend = None
            for p in range(NPAIR):
                nxt = s1_pair(0, p)
                if pend is not None:
                    ctx_mm(0, *pend)
                pend = nxt
            ctx_mm(0, *pend)
            mid_v(0)
            pend = None
            for p in range(NPAIR):
                nxt = s1_pair(1, p)
                if pend is not None:
                    ctx_mm(1, *pend)
                pend = nxt
                if p == 1:
                    mid_pe(0)
            ctx_mm(1, *pend)
            mid_v(1)
            apend = phaseA_front(0, 0)
            mid_pe(1)
            for s in range(S_PER_CORE):
                for b in range(NB):
                    if (s, b) == (0, 0):
                        continue
                    nxt = phaseA_front(s, b)
                    phaseA_out(*apend)
                    apend = nxt
            phaseA_out(*apend)
    nc.compile()
    return nc


def _host_prep(Wk, bk, Wq, bq, Wv, bv, We, be, x):
    f = np.float32

    def chunkT8(w):
        wt = np.ascontiguousarray((w.astype(np.float64) * WS).astype(f).T)
        r = wt.reshape(4, 128, w.shape[0]).transpose(1, 0, 2)
        return np.ascontiguousarray(r).astype(F8NP)

    def chunkT(w):
        wt = np.ascontiguousarray(w.astype(f).T)
        return np.ascontiguousarray(
            wt.reshape(4, 128, w.shape[0]).transpose(1, 0, 2))

    wk8 = chunkT8(Wk)
    wq8 = chunkT8(Wq)
    wv8 = chunkT8(Wv)
    weT = chunkT(We.astype(np.float64) / WS)

    ebqv = np.exp(bq.astype(np.float64))
    ebq2 = np.ascontiguousarray(ebqv.astype(f).reshape(2, 128).T)

    # exp(K) must stay under the fp8e4m3 cap (~240): estimate max K over a
    # strided sample of columns and shift the exponent if needed.
    xs = x[:, :, ::4].astype(f)
    wk8f = wk8.astype(f)  # quantized weights, chunk layout [128, 4, KC]
    wkq = (wk8f.transpose(1, 0, 2).reshape(C, KC).T / WS).astype(f)
    kmax = max(float(np.abs(wkq @ xs[n]).max()) for n in range(N))
    kshift = max(0.0, kmax + 0.4 - 5.4)
    ksh = np.full((128, 1), -kshift, f)

    # group-sum matrix with exp(bq) folded in per column, per j-half
    bsum = np.zeros((128, 2, 128), f)
    for p in range(128):
        g = (p // 32) * 32
        for j in range(2):
            bsum[p, j, g:g + 32] = ebqv[j * 128 + g:j * 128 + g + 32]

    wb = (We.astype(np.float64) @ bv.astype(np.float64)
          + be.astype(np.float64)).astype(f)

    shared = dict(wk8=wk8, wq8=wq8, wv8=wv8, weT=weT, ebq=ebq2, ksh=ksh,
                  bsum=bsum.astype(BFNP), ident=np.eye(128, dtype=f))
    return shared, wb


def kernel(x, Wk, bk, Wq, bq, Wv, bv, We, be):
    from concourse.bass_utils import run_bass_kernel_spmd

    assert x.shape == (N, C, Hdim, Wdim), x.shape
    if "nc" not in _CACHE:
        _CACHE["nc"] = _build_nc()
    nc = _CACHE["nc"]

    xf = np.ascontiguousarray(x.astype(np.float32).reshape(N, C, L))
    shared, wb = _host_prep(Wk, bk, Wq, bq, Wv, bv, We, be, xf)
    x8full = np.clip(xf, -240.0, 240.0).astype(F8NP)

    in_maps = []
    for i in range(N_CORES):
        m = dict(shared)
        m["x8"] = np.ascontiguousarray(
            x8full[i * S_PER_CORE:(i + 1) * S_PER_CORE]
            .reshape(S_PER_CORE * C, L))
        in_maps.append(m)

    res = run_bass_kernel_spmd(nc, in_maps, core_ids=list(range(N_CORES)))
    attn = np.concatenate(
        [res.results[i]["out"].astype(np.float32)
         .reshape(S_PER_CORE, C, L) for i in range(N_CORES)], axis=0)
    out = xf + attn + wb[None, :, None]
    return np.ascontiguousarray(
        out.reshape(N, C, Hdim, Wdim).astype(np.float32))
